# revision 9
# baseline (speedup 1.0000x reference)
"""Deformable cross-attention kernel for 8 Trainium2 NeuronCores.

Data-parallel over batch N=8: core i processes batch element i.
Per-core pipeline (v2 — SBUF-resident transposed value + ap_gather):
  1. host pre-transposed queryT -> offsets/attn projection (fp32 matmul)
  2. DVE weight math (q-partition layout): sampling locations, floors,
     clamps, border masks, softmax, parity-blend coefficients v0..v2,
     gather indices (r*64 + kh); coefficient planes PE-transposed to
     (h,pt)-partition layout (coefST)
  3. memory -> bf16 (cast in DMA) -> PE transpose -> fused W_value^T
     matmuls -> valueT [cout, pos] kept in SBUF (bf16, padded)
  4. coefST expanded to [h*32+ch, ...] via selection matmuls (host B)
  5. ap_gather (gpsimd extended inst, 8 Q7 cores): 16 calls of 2048
     idx, each fetching a 2-position pair per index from valueT
  6. DVE blend in [ch, q] layout; bias folded via host M = b_val @ W_out
     into the output projection; out = sampledT.T @ W_out + S.T @ M + b
"""
import json
import numpy as np
import ml_dtypes

N_B = 8
LQ = 1024
C = 256
NH = 8
NPT = 4
HD = 32
HW = 16384
GRID = 128  # H == W == 128
VPAD = 128  # pad columns on valueT so idx 8192 (w+1 overflow) stays in range


def _patch_compat(bass):
    """Split multi-wait instructions and sem-range-clears for this walrus."""
    if getattr(bass.Bass, "_dca_patched", False):
        return
    orig = bass.Bass.to_json_bytes

    def to_json_bytes(self):
        m = json.loads(orig(self))
        uid = 0
        sem_names = m.get("ant_sem_names") or {}
        for fn in m["functions"]:
            for bb in fn["blocks"]:
                out = []
                for inst in bb["instructions"]:
                    si = inst.get("sync_info")
                    waits = (si or {}).get("on_wait") or []
                    if len(waits) > 1:
                        for w in waits[:-1]:
                            uid += 1
                            out.append({
                                "debug": inst.get("debug", 0),
                                "engine": inst["engine"],
                                "ins": [], "outs": [],
                                "name": f"I-wsplit-{uid}",
                                "opcode": "EventSemaphore",
                                "sync_info": {"on_update": [], "on_wait": [w]},
                            })
                        si["on_wait"] = waits[-1:]
                    if (inst.get("opcode") == "ISA"
                            and inst.get("op_name") == "EVENT_SEMAPHORE_RANGE_CLEAR"):
                        d = inst["ant_dict"]
                        for sid in range(d["range_first"], d["range_last"] + 1):
                            uid += 1
                            out.append({
                                "debug": inst.get("debug", 0),
                                "engine": inst["engine"],
                                "ins": [], "outs": [],
                                "name": f"I-semclr-{uid}",
                                "opcode": "EventSemaphore",
                                "sync_info": {
                                    "on_wait": [],
                                    "on_update": [{
                                        "ant_name": sem_names.get(str(sid), f"sem{sid}"),
                                        "id": sid, "sync_type": "semaphore",
                                        "update_mode": "sem-wr-imm",
                                        "update_value": 0,
                                    }]},
                            })
                        continue
                    out.append(inst)
                bb["instructions"] = out
        return json.dumps(m).encode()

    bass.Bass.to_json_bytes = to_json_bytes
    bass.Bass._dca_patched = True


def _floor(nc, pool, mybir, x, shape, pfx):
    """Exact floor via double cast + is_gt correction (any cast rounding)."""
    F32 = mybir.dt.float32
    xi = pool.tile(shape, mybir.dt.int32, name=f"{pfx}_xi", tag=f"{pfx}_xi")
    nc.vector.tensor_copy(xi[:], x[:])
    xf = pool.tile(shape, F32, name=f"{pfx}_xf", tag=f"{pfx}_xf")
    nc.vector.tensor_copy(xf[:], xi[:])
    gt = pool.tile(shape, F32, name=f"{pfx}_gt", tag=f"{pfx}_gt")
    nc.vector.tensor_tensor(gt[:], xf[:], x[:], mybir.AluOpType.is_gt)
    fl = pool.tile(shape, F32, name=f"{pfx}_fl", tag=f"{pfx}_fl")
    nc.vector.tensor_tensor(fl[:], xf[:], gt[:], mybir.AluOpType.subtract)
    return fl


def build_program():
    import concourse.bass as bass
    import concourse.bacc as bacc
    import concourse.mybir as mybir
    import concourse.tile as tile
    from contextlib import ExitStack

    _patch_compat(bass)

    F32 = mybir.dt.float32
    F32R = mybir.dt.float32r
    BF16 = mybir.dt.bfloat16
    I16 = mybir.dt.int16
    TT = mybir.AluOpType
    ACTF = mybir.ActivationFunctionType

    nc = bacc.Bacc()

    # ---- external tensors ----
    qT_e = nc.declare_dram_parameter("qT", [C, LQ], F32, isOutput=False)
    memory_e = nc.declare_dram_parameter("memory", [HW, C], F32, isOutput=False)
    refpts_e = nc.declare_dram_parameter("refpts", [LQ, 2], F32, isOutput=False)
    w_value_e = nc.declare_dram_parameter("w_value", [2, 128, C], BF16, isOutput=False)
    w_oa_e = nc.declare_dram_parameter("w_oa", [C, 96], F32, isOutput=False)
    b_oa_e = nc.declare_dram_parameter("b_oa", [128, 96], F32, isOutput=False)
    w_out_e = nc.declare_dram_parameter("w_out", [2, 128, C], F32R, isOutput=False)
    b_out_e = nc.declare_dram_parameter("b_out", [128, C], F32, isOutput=False)
    m_fold_e = nc.declare_dram_parameter("m_fold", [NH, C], F32R, isOutput=False)
    bsel_e = nc.declare_dram_parameter("bsel", [8, 32, 128], BF16, isOutput=False)
    ident_e = nc.declare_dram_parameter("ident", [128, 128], F32, isOutput=False)
    identb_e = nc.declare_dram_parameter("identb", [128, 128], BF16, isOutput=False)
    out_e = nc.declare_dram_parameter("out", [LQ, C], F32, isOutput=True)

    with tile.TileContext(nc) as tc, ExitStack() as ctx:
        cpool = ctx.enter_context(tc.tile_pool(name="const", bufs=1))
        psA = ctx.enter_context(tc.tile_pool(name="psA", bufs=2, space="PSUM"))
        psB = ctx.enter_context(tc.tile_pool(name="psB", bufs=2, space="PSUM"))
        _n = [0]

        def ps_tr():  # [128,128] f32 transpose target
            _n[0] += 1
            return psA.tile([128, 128], F32, name=f"pstr{_n[0]}", tag="pstr")

        def ps_trb():  # bf16 transpose target
            _n[0] += 1
            return psA.tile([128, 128], BF16, name=f"psb{_n[0]}", tag="psb")

        def ps_mm512():  # [128,512] f32 matmul target
            _n[0] += 1
            return psB.tile([128, 512], F32, name=f"ps5{_n[0]}", tag="ps5")

        def ps_mm256():  # [128,256] f32 matmul target
            _n[0] += 1
            return psB.tile([128, C], F32, name=f"ps2{_n[0]}", tag="ps2")

        _cp = [0]

        def copy_ps(out_ap, in_ap):
            # alternate PSUM->SBUF copies between scalar and vector engines
            _cp[0] += 1
            if _cp[0] % 2 == 0:
                nc.scalar.copy(out_ap, in_ap)
            else:
                nc.vector.tensor_copy(out_ap, in_ap)

        ident = cpool.tile([128, 128], F32)
        nc.sync.dma_start(ident[:], ident_e[:])
        identb = cpool.tile([128, 128], BF16)
        nc.sync.dma_start(identb[:], identb_e[:])

        # ---------- 1. queryT load + offsets/attn projection ----------
        prep_ctx = tc.tile_pool(name="prep", bufs=1)
        prep = prep_ctx.__enter__()
        qT = prep.tile([128, 2, LQ], F32)
        nc.sync.dma_start(qT[:], qT_e[:].rearrange("(k p) q -> p k q", k=2))

        w_oa = prep.tile([128, 2, 96], F32)
        nc.sync.dma_start(w_oa[:], w_oa_e[:].rearrange("(k p) o -> p k o", k=2))
        b_oa = prep.tile([128, 96], F32)
        nc.sync.dma_start(b_oa[:], b_oa_e[:])

        off_all = prep.tile([128, 8, 96], F32)
        for qc in range(8):
            po = ps_mm256()
            for kc in range(2):
                nc.tensor.matmul(po[:, 0:96], qT[:, kc, qc * 128:(qc + 1) * 128],
                                 w_oa[:, kc, :], start=(kc == 0), stop=(kc == 1))
            nc.vector.tensor_tensor(off_all[:, qc, :], po[:, 0:96], b_oa[:], TT.add)

        refs = prep.tile([128, 8, 2], F32)
        nc.sync.dma_start(
            refs[:], refpts_e[:].rearrange("(g p) t -> p g t", p=128))

        # persistent q-side products (consumed after wmath scope closes)
        coefQ = prep.tile([128, 2, 3, 8, 8, 4], BF16)
        sfac = prep.tile([128, 8, 8], F32)
        idxf = prep.tile([128, 8, 8, 4, 2], F32)

        # ---------- 2. weight math (q-partition layout, scoped pool) ------
        S8 = [128, 8, 32]      # (q%128, qtop, (h, pt))

        def view_off(comp):  # comp 0 = x, 1 = y -> [128, 8, 8, 4] strided view
            return off_all[:, :, comp:64 + comp].rearrange(
                "p g (h pt two) -> p g h pt two", h=8, two=2)[:, :, :, :, 0]

        with tc.tile_pool(name="wmath", bufs=1) as wm:
            _wm_n = [0]

            def ttile():
                _wm_n[0] += 1
                nm = f"wmath{_wm_n[0]}"
                return wm.tile(S8, F32, name=nm, tag=nm)

            # --- x chain ---
            px = ttile()
            nc.vector.tensor_scalar(px[:], view_off(0), 1.0 / GRID, None, TT.mult)
            nc.vector.tensor_tensor(
                px[:], px[:], refs[:, :, 0:1].broadcast_to(S8), TT.add)
            nc.vector.tensor_scalar(px[:], px[:], float(GRID), -0.5, TT.mult, TT.add)
            x0 = _floor(nc, wm, mybir, px, S8, "fx0")
            wx1 = ttile()
            nc.vector.tensor_tensor(wx1[:], px[:], x0[:], TT.subtract)
            wx0 = ttile()
            nc.vector.tensor_scalar(wx0[:], wx1[:], -1.0, 1.0, TT.mult, TT.add)
            ge0 = ttile()
            nc.vector.tensor_scalar(ge0[:], x0[:], 0.0, None, TT.is_ge)
            le127 = ttile()
            nc.vector.tensor_scalar(le127[:], x0[:], 127.0, None, TT.is_le)
            le126 = ttile()
            nc.vector.tensor_scalar(le126[:], x0[:], 126.0, None, TT.is_le)
            eqm1 = ttile()
            nc.vector.tensor_scalar(eqm1[:], x0[:], -1.0, None, TT.is_equal)
            c0 = ttile()
            nc.vector.tensor_tensor(c0[:], ge0[:], le127[:], TT.mult)
            nc.vector.tensor_tensor(c0[:], c0[:], wx0[:], TT.mult)
            t_ = ttile()
            nc.vector.tensor_tensor(t_[:], wx1[:], eqm1[:], TT.mult)
            nc.vector.tensor_tensor(c0[:], c0[:], t_[:], TT.add)
            c1 = ttile()
            nc.vector.tensor_tensor(c1[:], ge0[:], le126[:], TT.mult)
            nc.vector.tensor_tensor(c1[:], c1[:], wx1[:], TT.mult)
            xs = ttile()
            nc.vector.tensor_scalar(xs[:], x0[:], 0.0, 127.0, TT.max, TT.min)
            xh = ttile()
            nc.vector.tensor_scalar(xh[:], xs[:], 0.5, None, TT.mult)
            kh = _floor(nc, wm, mybir, xh, S8, "fkh")
            spar = ttile()
            nc.vector.scalar_tensor_tensor(spar[:], kh[:], -2.0, xs[:], TT.mult, TT.add)

            # --- y chain ---
            py = ttile()
            nc.vector.tensor_scalar(py[:], view_off(1), 1.0 / GRID, None, TT.mult)
            nc.vector.tensor_tensor(
                py[:], py[:], refs[:, :, 1:2].broadcast_to(S8), TT.add)
            nc.vector.tensor_scalar(py[:], py[:], float(GRID), -0.5, TT.mult, TT.add)
            y0 = _floor(nc, wm, mybir, py, S8, "fy0")
            wy1 = ttile()
            nc.vector.tensor_tensor(wy1[:], py[:], y0[:], TT.subtract)
            wy0m = ttile()
            nc.vector.tensor_scalar(wy0m[:], wy1[:], -1.0, 1.0, TT.mult, TT.add)
            yge0 = ttile()
            nc.vector.tensor_scalar(yge0[:], y0[:], 0.0, None, TT.is_ge)
            yle127 = ttile()
            nc.vector.tensor_scalar(yle127[:], y0[:], 127.0, None, TT.is_le)
            nc.vector.tensor_tensor(yge0[:], yge0[:], yle127[:], TT.mult)
            nc.vector.tensor_tensor(wy0m[:], wy0m[:], yge0[:], TT.mult)
            ygem1 = ttile()
            nc.vector.tensor_scalar(ygem1[:], y0[:], -1.0, None, TT.is_ge)
            yle126 = ttile()
            nc.vector.tensor_scalar(yle126[:], y0[:], 126.0, None, TT.is_le)
            nc.vector.tensor_tensor(ygem1[:], ygem1[:], yle126[:], TT.mult)
            nc.vector.tensor_tensor(wy1[:], wy1[:], ygem1[:], TT.mult)
            r0 = ttile()
            nc.vector.tensor_scalar(r0[:], y0[:], 0.0, 127.0, TT.max, TT.min)
            r1 = ttile()
            nc.vector.tensor_scalar(r1[:], y0[:], 1.0, None, TT.add)
            nc.vector.tensor_scalar(r1[:], r1[:], 0.0, 127.0, TT.max, TT.min)

            # --- softmax over pt ---
            logit4 = off_all[:, :, 64:96].rearrange("p g (h pt) -> p g h pt", pt=4)
            mx = wm.tile([128, 8, 8], F32, name="smx", tag="smx")
            nc.vector.tensor_reduce(mx[:], logit4, mybir.AxisListType.X, TT.max)
            ee = ttile()
            nc.vector.tensor_tensor(
                ee[:].rearrange("p g (h pt) -> p g h pt", pt=4), logit4,
                mx[:].unsqueeze(3).broadcast_to([128, 8, 8, 4]),
                TT.subtract)
            nc.scalar.activation(ee[:], ee[:], ACTF.Exp)
            ssum = wm.tile([128, 8, 8], F32, name="ssum", tag="ssum")
            nc.vector.tensor_reduce(
                ssum[:], ee[:].rearrange("p g (h pt) -> p g h pt", pt=4),
                mybir.AxisListType.X, TT.add)
            rec = wm.tile([128, 8, 8], F32, name="srec", tag="srec")
            nc.vector.reciprocal(rec[:], ssum[:])
            attn = ttile()
            nc.vector.tensor_tensor(
                attn[:].rearrange("p g (h pt) -> p g h pt", pt=4),
                ee[:].rearrange("p g (h pt) -> p g h pt", pt=4),
                rec[:].unsqueeze(3).broadcast_to([128, 8, 8, 4]),
                TT.mult)

            # --- parity coefficients ---
            p0 = ttile()
            nc.vector.tensor_scalar(p0[:], spar[:], 0.0, None, TT.is_equal)
            p1 = ttile()
            nc.vector.tensor_scalar(p1[:], spar[:], 1.0, None, TT.is_equal)
            g0 = ttile()
            nc.vector.tensor_tensor(g0[:], attn[:], wy0m[:], TT.mult)
            g1 = ttile()
            nc.vector.tensor_tensor(g1[:], attn[:], wy1[:], TT.mult)
            v0 = ttile()
            nc.vector.tensor_tensor(v0[:], c0[:], p0[:], TT.mult)
            v1 = ttile()
            nc.vector.tensor_tensor(v1[:], c0[:], p1[:], TT.mult)
            nc.vector.tensor_tensor(t_[:], c1[:], p0[:], TT.mult)
            nc.vector.tensor_tensor(v1[:], v1[:], t_[:], TT.add)
            v2 = ttile()
            nc.vector.tensor_tensor(v2[:], c1[:], p1[:], TT.mult)
            for yi, gy in ((0, g0), (1, g1)):
                for ji, vj in ((0, v0), (1, v1), (2, v2)):
                    nc.vector.tensor_tensor(
                        coefQ[:, yi, ji, :, :, :],
                        gy[:].rearrange("p g (h pt) -> p g h pt", pt=4),
                        vj[:].rearrange("p g (h pt) -> p g h pt", pt=4), TT.mult)

            # bias-fold factor S[q, h]
            wys = ttile()
            nc.vector.tensor_tensor(wys[:], wy0m[:], wy1[:], TT.add)
            cxs = ttile()
            nc.vector.tensor_tensor(cxs[:], c0[:], c1[:], TT.add)
            nc.vector.tensor_tensor(wys[:], wys[:], cxs[:], TT.mult)
            nc.vector.tensor_tensor(wys[:], wys[:], attn[:], TT.mult)
            nc.vector.tensor_reduce(
                sfac[:], wys[:].rearrange("p g (h pt) -> p g h pt", pt=4),
                mybir.AxisListType.X, TT.add)

            # --- gather indices idxf = r*64 + kh ---
            for yi, rr in ((0, r0), (1, r1)):
                nc.vector.scalar_tensor_tensor(
                    idxf[:, :, :, :, yi],
                    rr[:].rearrange("p g (h pt) -> p g h pt", pt=4), 64.0,
                    kh[:].rearrange("p g (h pt) -> p g h pt", pt=4),
                    TT.mult, TT.add)

        # ---------- idx transform: [128, 512] -> idx16k ----------
        tall = [prep.tile([128, 128], F32, name=f"tall{t}", tag=f"tall{t}")
                for t in range(4)]
        idxf_flat = idxf[:].rearrange("p g h pt y -> p (g h pt y)")
        for t in range(4):
            pt_ = ps_tr()
            nc.tensor.transpose(
                pt_[:], idxf_flat[:, t * 128:(t + 1) * 128], ident[:])
            nc.scalar.copy(tall[t][:], pt_[:])
        idx_f2 = prep.tile([16, 64, 8, 8], I16)  # [qlo16, c, g, qmid]
        for t in range(4):
            for qmid in range(8):
                ptf = ps_tr()
                pt_ = ptf[0:16, :]
                nc.tensor.transpose(
                    pt_, tall[t][:, qmid * 16:qmid * 16 + 16], ident[:])
                src = pt_.rearrange("a (q2 c) -> a q2 c", q2=2)
                nc.scalar.copy(
                    idx_f2[:, :, 2 * t:2 * t + 2, qmid].rearrange(
                        "a c q2 -> a q2 c"), src)
        # idx16k [16, 2k, 64c, 64(g,qmid)] i16: k=0 -> idx, k=1 -> idx+1
        idx16k = prep.tile([16, 2, 64, 64], I16)
        idx_src = idx_f2[:].rearrange("a c g q -> a c (g q)")
        nc.vector.tensor_copy(idx16k[:, 0], idx_src)
        nc.vector.tensor_scalar(idx16k[:, 1], idx_src, 1, None, TT.add)
        # idxGall [128, k2, ps2, pt4, y2, 64(g,qm)] i16 via 8 block DMAs
        idxGall = cpool.tile([128, 2, 2, 4, 2, 64], I16)
        idx16v = idx16k[:].rearrange(
            "a k (hh hl pt y) (g qm) -> a k hh hl (pt y g qm)",
            hh=2, hl=4, pt=4, g=8)
        for b in range(8):
            src = idx16v[:, :, :, b // 2]
            nc.sync.dma_start(
                idxGall[b * 16:(b + 1) * 16].rearrange(
                    "a k ps pt y gq -> a k ps (pt y gq)"), src)

        # ---------- coefST: PE transpose coefQ -> [32 (h,pt), 2y, 3j, q] --
        coefST = cpool.tile([32, 2, 3, 8, 128], BF16)
        for yi in range(2):
            for ji in range(3):
                for g in range(8):
                    pt_ = ps_trb()
                    po = pt_[0:32, :]
                    nc.tensor.transpose(
                        po, coefQ[:, yi, ji, g, :, :].rearrange(
                            "p h pt -> p (h pt)"), identb[:])
                    nc.scalar.copy(coefST[:, yi, ji, g, :], po)

        # sfacT [8h, 8g, 128] f32 via PE transposes
        sfacT = cpool.tile([8, 8, 128], F32R)
        for g in range(8):
            pt_ = ps_tr()
            po = pt_[0:8, :]
            nc.tensor.transpose(po, sfac[:, g, :], ident[:])
            nc.scalar.copy(sfacT[:, g, :], po)

        prep_ctx.__exit__(None, None, None)
        post = ctx.enter_context(tc.tile_pool(name="post", bufs=1))
        sampledT = [post.tile([128, LQ], F32R, name=f"sT{i}", tag=f"sT{i}")
                    for i in range(2)]
        sacc = [post.tile([128, LQ], F32, name=f"sacc{i}", tag=f"sacc{i}")
                for i in range(2)]

        # ---------- 3. value projection -> valueT in SBUF ----------
        w_val = cpool.tile([128, 2, C], BF16)
        nc.sync.dma_start(w_val[:], w_value_e[:].rearrange("k p o -> p k o"))
        valueT = [cpool.tile([128, HW + VPAD], BF16, name=f"vT{i}", tag=f"vT{i}")
                  for i in range(2)]
        for i in range(2):
            nc.vector.memset(valueT[i][:, HW:], 0.0)

        with tc.tile_pool(name="mem", bufs=3) as mpool, \
             tc.tile_pool(name="mTg", bufs=3) as mtpool:
            for mc8 in range(16):
                mt = mpool.tile([128, 8, C], BF16)
                nc.gpsimd.dma_start(
                    mt[:], memory_e[mc8 * 1024:(mc8 + 1) * 1024, :].rearrange(
                        "(i p) c -> p i c", p=128))
                for sub in range(2):  # 512-position group
                    grp = mc8 * 2 + sub
                    mTg = mtpool.tile([128, 2, 512], BF16, tag="mTg")
                    for i in range(4):
                        for kc in range(2):
                            pt_ = ps_trb()
                            nc.tensor.transpose(
                                pt_[:],
                                mt[:, sub * 4 + i, kc * 128:(kc + 1) * 128],
                                identb[:])
                            copy_ps(mTg[:, kc, i * 128:(i + 1) * 128], pt_[:])
                    for half in range(2):
                        pv = ps_mm512()
                        for kc in range(2):
                            nc.tensor.matmul(
                                pv[:], w_val[:, kc, half * 128:(half + 1) * 128],
                                mTg[:, kc, :],
                                start=(kc == 0), stop=(kc == 1))
                        copy_ps(valueT[half][:, grp * 512:(grp + 1) * 512], pv[:])

        # ---------- 4/5. selection-expand coef + gather + blend ----------
        bsel = cpool.tile([32, 8, 128], BF16)
        nc.sync.dma_start(bsel[:], bsel_e[:].rearrange("b k m -> k b m"))

        gpool = ctx.enter_context(tc.tile_pool(name="gat", bufs=2))
        xpool = ctx.enter_context(tc.tile_pool(name="cfx", bufs=2))
        in_ap = [valueT[i][:].rearrange("p (n d) -> p n d", d=2)
                 for i in range(2)]
        NE = (HW + VPAD) // 2
        cflat = coefST[:].rearrange("k y j g q -> k (y j g q)")

        for ps in range(2):
            for pt in range(4):
                bidx = ps * 4 + pt
                coefX = xpool.tile([128, 2, 3, LQ], BF16, tag="coefX")
                cx_flat = coefX[:].rearrange("p y j q -> p (y j q)")
                for chunk in range(12):
                    pc = ps_mm512()
                    nc.tensor.matmul(
                        pc[:], bsel[:, bidx, :],
                        cflat[:, chunk * 512:(chunk + 1) * 512],
                        start=True, stop=True)
                    copy_ps(cx_flat[:, chunk * 512:(chunk + 1) * 512], pc[:])
                gat = [gpool.tile([128, 2048, 2], BF16, name=f"gat{k}",
                                  tag=f"gat{k}")
                       for k in range(2)]
                for k in range(2):
                    nc.gpsimd.ap_gather(
                        gat[k][:], in_ap[ps],
                        idxGall[:, k, ps, pt].rearrange("p y gq -> p (y gq)"),
                        128, NE, 2, 2048)
                sc = gpool.tile([128, LQ, 2, 3], BF16, tag="sc")
                gv0 = gat[0][:].rearrange("p (y q) x -> p q y x", y=2)
                nc.vector.tensor_tensor(
                    sc[:, :, :, 0:2], gv0,
                    coefX[:].rearrange("p y j q -> p q y j")[:, :, :, 0:2],
                    TT.mult)
                gv1 = gat[1][:].rearrange("p (y q) x -> p q y x", y=2)[:, :, :, 0]
                nc.vector.tensor_tensor(
                    sc[:, :, :, 2], gv1,
                    coefX[:].rearrange("p y j q -> p q y j")[:, :, :, 2],
                    TT.mult)
                if pt == 0:
                    nc.vector.tensor_reduce(
                        sacc[ps][:], sc[:], mybir.AxisListType.XY, TT.add)
                else:
                    red = gpool.tile([128, LQ], F32, tag="red")
                    nc.vector.tensor_reduce(
                        red[:], sc[:], mybir.AxisListType.XY, TT.add)
                    nc.vector.tensor_tensor(
                        sacc[ps][:], sacc[ps][:], red[:], TT.add)
            nc.scalar.copy(sampledT[ps][:], sacc[ps][:])

        # ---------- 6. output projection ----------
        w_out = cpool.tile([128, 2, C], F32R)
        nc.sync.dma_start(w_out[:], w_out_e[:].rearrange("k p o -> p k o"))
        m_fold = cpool.tile([NH, C], F32R)
        nc.sync.dma_start(m_fold[:], m_fold_e[:])
        b_out = cpool.tile([128, C], F32)
        nc.sync.dma_start(b_out[:], b_out_e[:])
        ost = [cpool.tile([128, 4, C], F32, name=f"ost{i}", tag=f"ost{i}")
               for i in range(2)]
        sfac_flat = sfacT[:].rearrange("h g q -> h (g q)")
        for qb in range(8):
            po = ps_mm256()
            nc.tensor.matmul(
                po[:], sampledT[0][:, qb * 128:(qb + 1) * 128],
                w_out[:, 0, :], start=True, stop=False)
            nc.tensor.matmul(
                po[:], sampledT[1][:, qb * 128:(qb + 1) * 128],
                w_out[:, 1, :], start=False, stop=False)
            nc.tensor.matmul(
                po[:], sfac_flat[:, qb * 128:(qb + 1) * 128],
                m_fold[:], start=False, stop=True)
            nc.vector.tensor_tensor(ost[qb // 4][:, qb % 4, :], po[:], b_out[:],
                                    TT.add)
        for i in range(2):
            nc.sync.dma_start(
                out_e[i * 512:(i + 1) * 512, :].rearrange(
                    "(j p) c -> p j c", p=128), ost[i][:])

    nc.finalize()
    return nc


_CACHE = {}


def _get_program():
    if "nc" not in _CACHE:
        _CACHE["nc"] = build_program()
    return _CACHE["nc"]


def _host_prep(inputs):
    query = np.asarray(inputs["query"], np.float32)
    memory = np.asarray(inputs["memory"], np.float32)
    refpts = np.asarray(inputs["reference_points"], np.float32)
    w_value = np.asarray(inputs["W_value"], np.float32)
    b_value = np.asarray(inputs["b_value"], np.float32)
    w_off = np.asarray(inputs["W_off"], np.float32)
    b_off = np.asarray(inputs["b_off"], np.float32)
    w_attn = np.asarray(inputs["W_attn"], np.float32)
    b_attn = np.asarray(inputs["b_attn"], np.float32)
    w_out = np.asarray(inputs["W_out"], np.float32)
    b_out = np.asarray(inputs["b_out"], np.float32)

    w_oa = np.concatenate([w_off, w_attn], axis=1).astype(np.float32)
    b_oa = np.tile(np.concatenate([b_off, b_attn])[None, :], (128, 1)).astype(
        np.float32)
    b_out_r = np.tile(b_out[None, :], (128, 1)).astype(np.float32)
    w_val_r = np.ascontiguousarray(w_value.reshape(2, 128, C)).astype(
        ml_dtypes.bfloat16)
    w_out_r = np.ascontiguousarray(w_out.reshape(2, 128, C))
    m_fold = np.einsum("hc,hco->ho", b_value.reshape(NH, HD),
                       w_out.reshape(NH, HD, C)).astype(np.float32)
    bsel = np.zeros((8, 32, 128), np.float32)
    for ps in range(2):
        for pt in range(4):
            for h4 in range(4):
                bsel[ps * 4 + pt, (ps * 4 + h4) * 4 + pt,
                     h4 * 32:(h4 + 1) * 32] = 1.0
    bsel = bsel.astype(ml_dtypes.bfloat16)
    ident = np.eye(128, dtype=np.float32)
    identb = np.eye(128, dtype=ml_dtypes.bfloat16)

    shared = dict(w_value=w_val_r, w_oa=w_oa, b_oa=b_oa, w_out=w_out_r,
                  b_out=b_out_r, m_fold=m_fold, bsel=bsel, ident=ident,
                  identb=identb)
    in_maps = []
    for i in range(N_B):
        m = dict(shared)
        m["qT"] = np.ascontiguousarray(query[i].T)
        m["memory"] = memory[i]
        m["refpts"] = refpts[i]
        in_maps.append(m)
    return in_maps


def run(inputs, trace=False):
    from concourse.bass_utils import run_bass_kernel_spmd

    nc = _get_program()
    in_maps = _host_prep(inputs)
    res = run_bass_kernel_spmd(nc, in_maps, list(range(N_B)), trace=trace,
                               trace_cores=[0])
    out = np.stack([res.results[i]["out"] for i in range(N_B)], axis=0)
    return out, res


def kernel(**inputs):
    assert int(inputs.get("H", GRID)) == GRID and int(inputs.get("W", GRID)) == GRID
    out, _ = run(inputs, trace=False)
    return out.astype(np.float32)


# revision 16
# speedup vs baseline: 1.2872x; 1.2872x over previous
"""Deformable cross-attention kernel for 8 Trainium2 NeuronCores.

Data-parallel over batch N=8: core i processes batch element i.
Per-core pipeline:
  1. query -> PE transpose -> offsets/attn projection (fp32 matmul)
  2. DVE weight math: sampling locations, floors, clamps, border masks,
     softmax, fused bilinear*attn coefficients, gather window indices
  3. memory -> bf16 (cast in DMA) -> PE transpose -> bf16 matmul with
     W_value -> value stored in DRAM as head-pairs [4][16386, 64] bf16
  4. dma_gather: 64 calls (head, point, ycorner) of 1024 query indices,
     overlapping 4-row windows (512B elems, 256B step)
  5. DVE blend: gathered * coeff (3 live sub-rows), reduce, accumulate
  6. bias folding, PE transpose of sampled, f32r output projection
"""
import json
import os
import numpy as np
import ml_dtypes

N_B = 8
LQ = 1024
C = 256
NH = 8
NPT = 4
HD = 32
HW = 16384
GRID = 128  # H == W == 128
PAIR_ROWS = HW + 2  # 2 pad rows, zeroed


def _patch_compat(bass):
    """Split multi-wait instructions and sem-range-clears for this walrus."""
    if getattr(bass.Bass, "_dca_patched", False):
        return
    orig = bass.Bass.to_json_bytes

    def to_json_bytes(self):
        m = json.loads(orig(self))
        uid = 0
        sem_names = m.get("ant_sem_names") or {}
        for fn in m["functions"]:
            for bb in fn["blocks"]:
                out = []
                for inst in bb["instructions"]:
                    si = inst.get("sync_info")
                    waits = (si or {}).get("on_wait") or []
                    if len(waits) > 1:
                        for w in waits[:-1]:
                            uid += 1
                            out.append({
                                "debug": inst.get("debug", 0),
                                "engine": inst["engine"],
                                "ins": [], "outs": [],
                                "name": f"I-wsplit-{uid}",
                                "opcode": "EventSemaphore",
                                "sync_info": {"on_update": [], "on_wait": [w]},
                            })
                        si["on_wait"] = waits[-1:]
                    if (inst.get("opcode") == "ISA"
                            and inst.get("op_name") == "EVENT_SEMAPHORE_RANGE_CLEAR"):
                        d = inst["ant_dict"]
                        for sid in range(d["range_first"], d["range_last"] + 1):
                            uid += 1
                            out.append({
                                "debug": inst.get("debug", 0),
                                "engine": inst["engine"],
                                "ins": [], "outs": [],
                                "name": f"I-semclr-{uid}",
                                "opcode": "EventSemaphore",
                                "sync_info": {
                                    "on_wait": [],
                                    "on_update": [{
                                        "ant_name": sem_names.get(str(sid), f"sem{sid}"),
                                        "id": sid, "sync_type": "semaphore",
                                        "update_mode": "sem-wr-imm",
                                        "update_value": 0,
                                    }]},
                            })
                        continue
                    out.append(inst)
                bb["instructions"] = out
        return json.dumps(m).encode()

    bass.Bass.to_json_bytes = to_json_bytes
    bass.Bass._dca_patched = True


def _floor(nc, pool, mybir, x, shape, pfx):
    """Exact floor via double cast + is_gt correction (any cast rounding)."""
    F32 = mybir.dt.float32
    xi = pool.tile(shape, mybir.dt.int32, name=f"{pfx}_xi", tag=f"{pfx}_xi")
    nc.vector.tensor_copy(xi[:], x[:])
    xf = pool.tile(shape, F32, name=f"{pfx}_xf", tag=f"{pfx}_xf")
    nc.vector.tensor_copy(xf[:], xi[:])
    gt = pool.tile(shape, F32, name=f"{pfx}_gt", tag=f"{pfx}_gt")
    nc.vector.tensor_tensor(gt[:], xf[:], x[:], mybir.AluOpType.is_gt)
    fl = pool.tile(shape, F32, name=f"{pfx}_fl", tag=f"{pfx}_fl")
    nc.vector.tensor_tensor(fl[:], xf[:], gt[:], mybir.AluOpType.subtract)
    return fl


def build_program():
    import concourse.bass as bass
    import concourse.bacc as bacc
    import concourse.mybir as mybir
    import concourse.tile as tile
    from contextlib import ExitStack

    _patch_compat(bass)

    F32 = mybir.dt.float32
    F32R = mybir.dt.float32r
    BF16 = mybir.dt.bfloat16
    I16 = mybir.dt.int16
    TT = mybir.AluOpType
    ACTF = mybir.ActivationFunctionType

    nc = bacc.Bacc()

    # ---- external tensors ----
    query_e = nc.declare_dram_parameter("query", [LQ, C], F32, isOutput=False)
    memory_e = nc.declare_dram_parameter("memory", [HW, C], F32, isOutput=False)
    refpts_e = nc.declare_dram_parameter("refpts", [LQ, 2], F32, isOutput=False)
    w_value_e = nc.declare_dram_parameter("w_value", [C, C], BF16, isOutput=False)
    w_oa_e = nc.declare_dram_parameter("w_oa", [C, 96], F32, isOutput=False)
    b_oa_e = nc.declare_dram_parameter("b_oa", [128, 96], F32, isOutput=False)
    w_out_e = nc.declare_dram_parameter("w_out", [C, C], F32R, isOutput=False)
    b_out_e = nc.declare_dram_parameter("b_out", [128, C], F32, isOutput=False)
    b_val_e = nc.declare_dram_parameter("b_val", [128, C], F32, isOutput=False)
    ident_e = nc.declare_dram_parameter("ident", [128, 128], F32, isOutput=False)
    identb_e = nc.declare_dram_parameter("identb", [128, 128], BF16, isOutput=False)
    out_e = nc.declare_dram_parameter("out", [LQ, C], F32, isOutput=True)

    # internal DRAM: value as head-pairs [4][PAIR_ROWS, 64] bf16
    value_d = nc.dram_tensor("value_pairs", [4, PAIR_ROWS, 64], BF16)

    with tile.TileContext(nc) as tc, ExitStack() as ctx:
        cpool = ctx.enter_context(tc.tile_pool(name="const", bufs=1))
        qpool = ctx.enter_context(tc.tile_pool(name="qp", bufs=2))
        wpool = ctx.enter_context(tc.tile_pool(name="wm", bufs=1))
        mpool = ctx.enter_context(tc.tile_pool(name="mem", bufs=3))
        vpool = ctx.enter_context(tc.tile_pool(name="val", bufs=3))
        gpool = ctx.enter_context(tc.tile_pool(name="gat", bufs=4))
        spool = ctx.enter_context(tc.tile_pool(name="scr", bufs=2))
        psA = ctx.enter_context(tc.tile_pool(name="psA", bufs=3, space="PSUM"))
        psB = ctx.enter_context(tc.tile_pool(name="psB", bufs=2, space="PSUM"))
        _ps_n = [0]

        def ps_tr():  # [128,128] f32 transpose target, shared slots
            _ps_n[0] += 1
            return psA.tile([128, 128], F32, name=f"pstr{_ps_n[0]}", tag="pstr")

        def ps_trb():  # bf16 transpose target
            _ps_n[0] += 1
            return psA.tile([128, 128], BF16, name=f"psb{_ps_n[0]}", tag="psb")

        def ps_mm():  # [128,256] f32 matmul target
            _ps_n[0] += 1
            return psB.tile([128, C], F32, name=f"psmm{_ps_n[0]}", tag="psmm")

        ident = cpool.tile([128, 128], F32)
        nc.sync.dma_start(ident[:], ident_e[:])
        identb = cpool.tile([128, 128], BF16)
        nc.sync.dma_start(identb[:], identb_e[:])

        # ---------- 1. queryT + offsets/attn projection ----------
        # query [1024, 256] -> queryT [2][128, 1024]
        qT = [cpool.tile([128, LQ], F32, name=f"qT{i}", tag=f"qT{i}")
              for i in range(2)]
        for qc in range(8):
            qt = qpool.tile([128, C], F32)
            nc.sync.dma_start(qt[:], query_e[qc * 128:(qc + 1) * 128, :])
            for kc in range(2):
                pt = ps_tr()
                nc.tensor.transpose(pt[:], qt[:, kc * 128:(kc + 1) * 128], ident[:])
                nc.scalar.copy(qT[kc][:, qc * 128:(qc + 1) * 128], pt[:])

        w_oa = cpool.tile([128, 2, 96], F32)
        nc.sync.dma_start(w_oa[:], w_oa_e[:].rearrange("(k p) o -> p k o", k=2))
        b_oa = cpool.tile([128, 96], F32)
        nc.sync.dma_start(b_oa[:], b_oa_e[:])

        # off_all [128, 8, 96] natural layout (partition = q%128, qtop free)
        off_all = cpool.tile([128, 8, 96], F32)
        for qc in range(8):
            po = ps_mm()
            for kc in range(2):
                nc.tensor.matmul(po[:, 0:96], qT[kc][:, qc * 128:(qc + 1) * 128],
                                 w_oa[:, kc, :], start=(kc == 0), stop=(kc == 1))
            nc.vector.tensor_tensor(off_all[:, qc, :], po[:, 0:96], b_oa[:], TT.add)

        # refpts natural [128, 8, 2]
        refs = cpool.tile([128, 8, 2], F32)
        nc.sync.dma_start(
            refs[:], refpts_e[:].rearrange("(g p) t -> p g t", p=128))

        # ---------- 2. weight math ----------
        S8 = [128, 8, 32]      # (q%128, qtop, (h, pt))

        def view_off(comp):  # comp 0 = x, 1 = y -> [128, 8, 8, 4] strided view
            return off_all[:, :, comp:64 + comp].rearrange(
                "p g (h pt two) -> p g h pt two", h=8, two=2)[:, :, :, :, 0]

        wm = ctx.enter_context(tc.tile_pool(name="wmath", bufs=1))

        _wm_n = [0]

        def ttile():
            _wm_n[0] += 1
            nm = f"wmath{_wm_n[0]}"
            return wm.tile(S8, F32, name=nm, tag=nm)

        # --- x chain ---
        px = ttile()
        # px = (ref_x + ox/128) * 128 - 0.5   (matches reference rounding)
        nc.vector.tensor_scalar(px[:], view_off(0), 1.0 / GRID, None, TT.mult)
        nc.vector.tensor_tensor(
            px[:], px[:], refs[:, :, 0:1].broadcast_to(S8), TT.add)
        nc.vector.tensor_scalar(px[:], px[:], float(GRID), -0.5, TT.mult, TT.add)
        x0 = _floor(nc, wm, mybir, px, S8, "fx0")
        wx1 = ttile()
        nc.vector.tensor_tensor(wx1[:], px[:], x0[:], TT.subtract)
        wx0 = ttile()
        nc.vector.tensor_scalar(wx0[:], wx1[:], -1.0, 1.0, TT.mult, TT.add)
        ge0 = ttile()
        nc.vector.tensor_scalar(ge0[:], x0[:], 0.0, None, TT.is_ge)
        le127 = ttile()
        nc.vector.tensor_scalar(le127[:], x0[:], 127.0, None, TT.is_le)
        le126 = ttile()
        nc.vector.tensor_scalar(le126[:], x0[:], 126.0, None, TT.is_le)
        eqm1 = ttile()
        nc.vector.tensor_scalar(eqm1[:], x0[:], -1.0, None, TT.is_equal)
        # c0 = wx0*inb(x0) + wx1*(x0 == -1);  c1 = wx1*(0 <= x0 <= 126)
        c0 = ttile()
        nc.vector.tensor_tensor(c0[:], ge0[:], le127[:], TT.mult)
        nc.vector.tensor_tensor(c0[:], c0[:], wx0[:], TT.mult)
        t_ = ttile()
        nc.vector.tensor_tensor(t_[:], wx1[:], eqm1[:], TT.mult)
        nc.vector.tensor_tensor(c0[:], c0[:], t_[:], TT.add)
        c1 = ttile()
        nc.vector.tensor_tensor(c1[:], ge0[:], le126[:], TT.mult)
        nc.vector.tensor_tensor(c1[:], c1[:], wx1[:], TT.mult)
        # xs = clip(x0, 0, 127); kh = floor(xs/2); s = xs - 2*kh
        xs = ttile()
        nc.vector.tensor_scalar(xs[:], x0[:], 0.0, 127.0, TT.max, TT.min)
        xh = ttile()
        nc.vector.tensor_scalar(xh[:], xs[:], 0.5, None, TT.mult)
        kh = _floor(nc, wm, mybir, xh, S8, "fkh")
        spar = ttile()
        nc.vector.scalar_tensor_tensor(spar[:], kh[:], -2.0, xs[:], TT.mult, TT.add)

        # --- y chain ---
        py = ttile()
        nc.vector.tensor_scalar(py[:], view_off(1), 1.0 / GRID, None, TT.mult)
        nc.vector.tensor_tensor(
            py[:], py[:], refs[:, :, 1:2].broadcast_to(S8), TT.add)
        nc.vector.tensor_scalar(py[:], py[:], float(GRID), -0.5, TT.mult, TT.add)
        y0 = _floor(nc, wm, mybir, py, S8, "fy0")
        wy1 = ttile()
        nc.vector.tensor_tensor(wy1[:], py[:], y0[:], TT.subtract)
        wy0m = ttile()
        nc.vector.tensor_scalar(wy0m[:], wy1[:], -1.0, 1.0, TT.mult, TT.add)
        yge0 = ttile()
        nc.vector.tensor_scalar(yge0[:], y0[:], 0.0, None, TT.is_ge)
        yle127 = ttile()
        nc.vector.tensor_scalar(yle127[:], y0[:], 127.0, None, TT.is_le)
        nc.vector.tensor_tensor(yge0[:], yge0[:], yle127[:], TT.mult)
        nc.vector.tensor_tensor(wy0m[:], wy0m[:], yge0[:], TT.mult)  # wy0*inb(y0)
        ygem1 = ttile()
        nc.vector.tensor_scalar(ygem1[:], y0[:], -1.0, None, TT.is_ge)
        yle126 = ttile()
        nc.vector.tensor_scalar(yle126[:], y0[:], 126.0, None, TT.is_le)
        nc.vector.tensor_tensor(ygem1[:], ygem1[:], yle126[:], TT.mult)
        nc.vector.tensor_tensor(wy1[:], wy1[:], ygem1[:], TT.mult)   # wy1*inb(y1)
        r0 = ttile()
        nc.vector.tensor_scalar(r0[:], y0[:], 0.0, 127.0, TT.max, TT.min)
        r1 = ttile()
        nc.vector.tensor_scalar(r1[:], y0[:], 1.0, None, TT.add)
        nc.vector.tensor_scalar(r1[:], r1[:], 0.0, 127.0, TT.max, TT.min)

        # --- softmax over pt ---
        logit4 = off_all[:, :, 64:96].rearrange("p g (h pt) -> p g h pt", pt=4)
        mx = wm.tile([128, 8, 8], F32, name="smx", tag="smx")
        nc.vector.tensor_reduce(mx[:], logit4, mybir.AxisListType.X, TT.max)
        ee = ttile()
        nc.vector.tensor_tensor(
            ee[:].rearrange("p g (h pt) -> p g h pt", pt=4), logit4,
            mx[:].unsqueeze(3).broadcast_to([128, 8, 8, 4]),
            TT.subtract)
        nc.scalar.activation(ee[:], ee[:], ACTF.Exp)
        ssum = wm.tile([128, 8, 8], F32, name="ssum", tag="ssum")
        nc.vector.tensor_reduce(
            ssum[:], ee[:].rearrange("p g (h pt) -> p g h pt", pt=4),
            mybir.AxisListType.X, TT.add)
        rec = wm.tile([128, 8, 8], F32, name="srec", tag="srec")
        nc.vector.reciprocal(rec[:], ssum[:])
        attn = ttile()
        nc.vector.tensor_tensor(
            attn[:].rearrange("p g (h pt) -> p g h pt", pt=4),
            ee[:].rearrange("p g (h pt) -> p g h pt", pt=4),
            rec[:].unsqueeze(3).broadcast_to([128, 8, 8, 4]),
            TT.mult)

        # --- fused coefficients C[y][j] = attn * wy_y_masked * v_j ---
        # v0 = c0*(s==0), v1 = c0*(s==1)+c1*(s==0), v2 = c1*(s==1)
        p0 = ttile()
        nc.vector.tensor_scalar(p0[:], spar[:], 0.0, None, TT.is_equal)
        p1 = ttile()
        nc.vector.tensor_scalar(p1[:], spar[:], 1.0, None, TT.is_equal)
        g0 = ttile()
        nc.vector.tensor_tensor(g0[:], attn[:], wy0m[:], TT.mult)
        g1 = ttile()
        nc.vector.tensor_tensor(g1[:], attn[:], wy1[:], TT.mult)
        v0 = ttile()
        nc.vector.tensor_tensor(v0[:], c0[:], p0[:], TT.mult)
        v1 = ttile()
        nc.vector.tensor_tensor(v1[:], c0[:], p1[:], TT.mult)
        nc.vector.tensor_tensor(t_[:], c1[:], p0[:], TT.mult)
        nc.vector.tensor_tensor(v1[:], v1[:], t_[:], TT.add)
        v2 = ttile()
        nc.vector.tensor_tensor(v2[:], c1[:], p1[:], TT.mult)
        # coeff tensor [128, qtop, y, j, h, pt] fp32
        coef = cpool.tile([128, 8, 2, 3, 8, 4], F32)
        for yi, gy in ((0, g0), (1, g1)):
            for ji, vj in ((0, v0), (1, v1), (2, v2)):
                nc.vector.tensor_tensor(
                    coef[:, :, yi, ji, :, :],
                    gy[:].rearrange("p g (h pt) -> p g h pt", pt=4),
                    vj[:].rearrange("p g (h pt) -> p g h pt", pt=4), TT.mult)

        # bias-fold factor S[q, h] = sum_pt attn*(wy0m+wy1m)*(c0+c1)
        wys = ttile()
        nc.vector.tensor_tensor(wys[:], wy0m[:], wy1[:], TT.add)
        cxs = ttile()
        nc.vector.tensor_tensor(cxs[:], c0[:], c1[:], TT.add)
        nc.vector.tensor_tensor(wys[:], wys[:], cxs[:], TT.mult)
        nc.vector.tensor_tensor(wys[:], wys[:], attn[:], TT.mult)
        sfac = cpool.tile([128, 8, 8], F32)
        nc.vector.tensor_reduce(
            sfac[:], wys[:].rearrange("p g (h pt) -> p g h pt", pt=4),
            mybir.AxisListType.X, TT.add)

        # --- gather window indices idxf [128, (qtop, h, pt, y)] fp32 ---
        idxf = cpool.tile([128, 8, 8, 4, 2], F32)
        for yi, rr in ((0, r0), (1, r1)):
            nc.vector.scalar_tensor_tensor(
                idxf[:, :, :, :, yi],
                rr[:].rearrange("p g (h pt) -> p g h pt", pt=4), 64.0,
                kh[:].rearrange("p g (h pt) -> p g h pt", pt=4),
                TT.mult, TT.add)

        # ---------- idx layout transform: [128, 512] -> [16, 64, 64] ----------
        # T1: 4 PE transposes -> T-all [4][128 f, 128 q%128]
        tall = [cpool.tile([128, 128], F32, name=f"tall{t}", tag=f"tall{t}")
                for t in range(4)]
        idxf_flat = idxf[:].rearrange("p g h pt y -> p (g h pt y)")
        for t in range(4):
            pt_ = ps_tr()
            nc.tensor.transpose(
                pt_[:], idxf_flat[:, t * 128:(t + 1) * 128], ident[:])
            nc.scalar.copy(tall[t][:], pt_[:])
        # T2: per (t, qmid): [128 f, 16] -> [16, 128 f]; scatter into IDX
        idx_f2 = cpool.tile([16, 64, 8, 8], F32)  # [qlo, call, qtop, qmid]
        for t in range(4):
            for qmid in range(8):
                ptf = ps_tr()
                pt_ = ptf[0:16, :]
                nc.tensor.transpose(
                    pt_, tall[t][:, qmid * 16:qmid * 16 + 16], ident[:])
                # f = t*128 + j, j = (qtop%2)*64 + call ; qtop = 2t + (j//64)
                src = pt_.rearrange("a (q2 c) -> a q2 c", q2=2)
                nc.scalar.copy(
                    idx_f2[:, :, 2 * t:2 * t + 2, qmid].rearrange(
                        "a c q2 -> a q2 c"), src)
        idx16 = cpool.tile([16, 64 * 64], I16)
        nc.vector.tensor_copy(
            idx16[:], idx_f2[:].rearrange("a c g q -> a (c g q)"))
        idxr = cpool.tile([128, 64, 64], I16)
        for rep in range(8):
            nc.sync.dma_start(
                idxr[rep * 16:(rep + 1) * 16, :, :],
                idx16[:].rearrange("a (c b) -> a c b", c=64))

        # ---------- 3. value projection ----------
        w_val = cpool.tile([128, 2, C], BF16)
        nc.sync.dma_start(w_val[:], w_value_e[:].rearrange(
            "(k p) o -> p k o", k=2))
        zpad = cpool.tile([2, 64], BF16)
        nc.vector.memset(zpad[:], 0.0)
        for pr in range(4):
            nc.sync.dma_start(value_d[pr, HW:HW + 2, :], zpad[:])

        for mc in range(128):
            mt = mpool.tile([128, C], BF16)
            nc.gpsimd.dma_start(mt[:], memory_e[mc * 128:(mc + 1) * 128, :])
            mT = mpool.tile([128, 2, 128], BF16, tag="mT")
            for kc in range(2):
                pt_ = ps_trb()
                nc.tensor.transpose(
                    pt_[:], mt[:, kc * 128:(kc + 1) * 128], identb[:])
                nc.scalar.copy(mT[:, kc, :], pt_[:])
            pv = ps_mm()
            for kc in range(2):
                nc.tensor.matmul(pv[:], mT[:, kc, :], w_val[:, kc, :],
                                 start=(kc == 0), stop=(kc == 1))
            vt = vpool.tile([128, C], BF16)
            nc.scalar.copy(vt[:], pv[:])
            for pr in range(4):
                eng = nc.sync if (mc * 4 + pr) % 2 == 0 else nc.scalar
                eng.dma_start(
                    value_d[pr, mc * 128:(mc + 1) * 128, :],
                    vt[:, pr * 64:(pr + 1) * 64])

        # ---------- 4 & 5. gather + blend ----------
        sampled = cpool.tile([128, 8, 8, 32], F32)  # [q%128, qtop, h, c]
        val_flat = value_d[:].rearrange("pr r c -> (pr r c)")
        for h in range(NH):
            pr = h // 2
            half = h % 2
            base = pr * (PAIR_ROWS * 64)
            in_ap = val_flat[base:base + 8192 * 128].rearrange(
                "(n c) -> n c", c=128).copy()
            in_ap.ap[-1] = (1, 256)  # overlapping 256-elem windows, step 128
            acc = spool.tile([128, 8, 32], F32, tag="acc")
            first = True
            for pt_i in range(NPT):
                for yi in range(2):
                    call = ((h * NPT) + pt_i) * 2 + yi
                    gat = gpool.tile([128, 8, 256], BF16)
                    nc.gpsimd.dma_gather(
                        gat[:], in_ap, idxr[:, call, :], LQ, LQ, 256,
                        elem_step=128)
                    sc = gpool.tile([128, 8, 3, 32], F32, tag="scaled")
                    g3 = gat[:].rearrange("p g (j c) -> p g j c", c=64)[
                        :, :, 0:3, half * 32:half * 32 + 32]
                    cf = coef[:, :, yi, :, h, pt_i].unsqueeze(3).broadcast_to([128, 8, 3, 32])
                    nc.vector.tensor_tensor(sc[:], g3, cf, TT.mult)
                    red = gpool.tile([128, 8, 32], F32, tag="red")
                    nc.vector.tensor_reduce(
                        red[:], sc[:].rearrange("p g j c -> p g c j"),
                        mybir.AxisListType.X, TT.add)
                    if first:
                        nc.vector.tensor_copy(acc[:], red[:])
                        first = False
                    else:
                        nc.vector.tensor_tensor(acc[:], acc[:], red[:], TT.add)
            nc.vector.tensor_copy(sampled[:, :, h, :], acc[:])

        # bias fold: sampled += S[q, h] * b_value[h*32 + c]
        b_val = cpool.tile([128, C], F32)
        nc.sync.dma_start(b_val[:], b_val_e[:])
        bterm = spool.tile([128, 8, 8, 32], F32, tag="bterm")
        nc.vector.tensor_tensor(
            bterm[:],
            sfac[:].unsqueeze(3).broadcast_to([128, 8, 8, 32]),
            b_val[:].rearrange("p (h c) -> p h c", h=8).unsqueeze(1).broadcast_to(
                [128, 8, 8, 32]),
            TT.mult)
        nc.vector.tensor_tensor(sampled[:], sampled[:], bterm[:], TT.add)

        # ---------- 6. output projection ----------
        # sampledT [2][128 hc, (qtop, q%128)] f32r
        sT = [cpool.tile([128, 8, 128], F32R, name=f"sT{i}", tag=f"sT{i}")
              for i in range(2)]
        for qt_ in range(8):
            for hf in range(2):
                pt_ = ps_tr()
                nc.tensor.transpose(
                    pt_[:],
                    sampled[:, qt_, hf * 4:(hf + 1) * 4, :].rearrange(
                        "p h c -> p (h c)"),
                    ident[:])
                nc.scalar.copy(sT[hf][:, qt_, :], pt_[:])
        w_out = cpool.tile([128, 2, C], F32R)
        nc.sync.dma_start(w_out[:], w_out_e[:].rearrange(
            "(k p) o -> p k o", k=2))
        b_out = cpool.tile([128, C], F32)
        nc.sync.dma_start(b_out[:], b_out_e[:])
        for qt_ in range(8):
            po = ps_mm()
            for kc in range(2):
                nc.tensor.matmul(po[:], sT[kc][:, qt_, :], w_out[:, kc, :],
                                 start=(kc == 0), stop=(kc == 1))
            ot = qpool.tile([128, C], F32, tag="out")
            nc.vector.tensor_tensor(ot[:], po[:], b_out[:], TT.add)
            nc.sync.dma_start(out_e[qt_ * 128:(qt_ + 1) * 128, :], ot[:])

    nc.finalize()
    return nc


_CACHE = {}


def _get_program():
    if "nc" not in _CACHE:
        _CACHE["nc"] = build_program()
    return _CACHE["nc"]


def run(inputs, trace=False):
    from concourse.bass_utils import run_bass_kernel_spmd

    nc = _get_program()
    query = np.asarray(inputs["query"], np.float32)
    memory = np.asarray(inputs["memory"], np.float32)
    refpts = np.asarray(inputs["reference_points"], np.float32)
    w_value = np.asarray(inputs["W_value"], np.float32).astype(ml_dtypes.bfloat16)
    b_value = np.asarray(inputs["b_value"], np.float32)
    w_off = np.asarray(inputs["W_off"], np.float32)
    b_off = np.asarray(inputs["b_off"], np.float32)
    w_attn = np.asarray(inputs["W_attn"], np.float32)
    b_attn = np.asarray(inputs["b_attn"], np.float32)
    w_out = np.asarray(inputs["W_out"], np.float32)
    b_out = np.asarray(inputs["b_out"], np.float32)

    w_oa = np.concatenate([w_off, w_attn], axis=1).astype(np.float32)
    b_oa = np.tile(np.concatenate([b_off, b_attn])[None, :], (128, 1)).astype(
        np.float32)
    b_out_r = np.tile(b_out[None, :], (128, 1)).astype(np.float32)
    b_val_r = np.tile(b_value[None, :], (128, 1)).astype(np.float32)
    ident = np.eye(128, dtype=np.float32)
    identb = np.eye(128, dtype=ml_dtypes.bfloat16)

    shared = dict(w_value=w_value, w_oa=w_oa, b_oa=b_oa, w_out=w_out,
                  b_out=b_out_r, b_val=b_val_r, ident=ident, identb=identb)
    in_maps = []
    for i in range(N_B):
        m = dict(shared)
        m["query"] = query[i]
        m["memory"] = memory[i]
        m["refpts"] = refpts[i]
        in_maps.append(m)

    res = run_bass_kernel_spmd(nc, in_maps, list(range(N_B)), trace=trace,
                               trace_cores=[0])
    out = np.stack([res.results[i]["out"] for i in range(N_B)], axis=0)
    return out, res


def kernel(**inputs):
    assert int(inputs.get("H", GRID)) == GRID and int(inputs.get("W", GRID)) == GRID
    out, _ = run(inputs, trace=False)
    return out.astype(np.float32)



# revision 17
# speedup vs baseline: 1.4747x; 1.1456x over previous
"""Deformable cross-attention kernel for 8 Trainium2 NeuronCores.

Data-parallel over batch N=8: core i processes batch element i.
Per-core pipeline:
  1. query -> PE transpose -> offsets/attn projection (fp32 matmul)
  2. DVE weight math: sampling locations, floors, clamps, border masks,
     softmax, fused bilinear*attn coefficients, gather window indices
  3. memory -> bf16 (cast in DMA) -> PE transpose -> bf16 matmul with
     W_value -> value stored in DRAM as head-pairs [4][16386, 64] bf16
  4. dma_gather: 64 calls (head, point, ycorner) of 1024 query indices,
     overlapping 4-row windows (512B elems, 256B step)
  5. DVE blend: gathered * coeff (3 live sub-rows), reduce, accumulate
  6. bias folding, PE transpose of sampled, f32r output projection
"""
import json
import os
import numpy as np
import ml_dtypes

N_B = 8
LQ = 1024
C = 256
NH = 8
NPT = 4
HD = 32
HW = 16384
GRID = 128  # H == W == 128
PAIR_ROWS = HW + 2  # 2 pad rows, zeroed


def _patch_compat(bass):
    """Split multi-wait instructions and sem-range-clears for this walrus."""
    if getattr(bass.Bass, "_dca_patched", False):
        return
    orig = bass.Bass.to_json_bytes

    def to_json_bytes(self):
        m = json.loads(orig(self))
        uid = 0
        sem_names = m.get("ant_sem_names") or {}
        for fn in m["functions"]:
            for bb in fn["blocks"]:
                out = []
                for inst in bb["instructions"]:
                    si = inst.get("sync_info")
                    waits = (si or {}).get("on_wait") or []
                    if len(waits) > 1:
                        for w in waits[:-1]:
                            uid += 1
                            out.append({
                                "debug": inst.get("debug", 0),
                                "engine": inst["engine"],
                                "ins": [], "outs": [],
                                "name": f"I-wsplit-{uid}",
                                "opcode": "EventSemaphore",
                                "sync_info": {"on_update": [], "on_wait": [w]},
                            })
                        si["on_wait"] = waits[-1:]
                    if (inst.get("opcode") == "ISA"
                            and inst.get("op_name") == "EVENT_SEMAPHORE_RANGE_CLEAR"):
                        d = inst["ant_dict"]
                        for sid in range(d["range_first"], d["range_last"] + 1):
                            uid += 1
                            out.append({
                                "debug": inst.get("debug", 0),
                                "engine": inst["engine"],
                                "ins": [], "outs": [],
                                "name": f"I-semclr-{uid}",
                                "opcode": "EventSemaphore",
                                "sync_info": {
                                    "on_wait": [],
                                    "on_update": [{
                                        "ant_name": sem_names.get(str(sid), f"sem{sid}"),
                                        "id": sid, "sync_type": "semaphore",
                                        "update_mode": "sem-wr-imm",
                                        "update_value": 0,
                                    }]},
                            })
                        continue
                    out.append(inst)
                bb["instructions"] = out
        return json.dumps(m).encode()

    bass.Bass.to_json_bytes = to_json_bytes
    bass.Bass._dca_patched = True


def _floor(nc, pool, mybir, x, shape, pfx):
    """Exact floor via double cast + is_gt correction (any cast rounding)."""
    F32 = mybir.dt.float32
    xi = pool.tile(shape, mybir.dt.int32, name=f"{pfx}_xi", tag=f"{pfx}_xi")
    nc.vector.tensor_copy(xi[:], x[:])
    xf = pool.tile(shape, F32, name=f"{pfx}_xf", tag=f"{pfx}_xf")
    nc.vector.tensor_copy(xf[:], xi[:])
    gt = pool.tile(shape, F32, name=f"{pfx}_gt", tag=f"{pfx}_gt")
    nc.vector.tensor_tensor(gt[:], xf[:], x[:], mybir.AluOpType.is_gt)
    fl = pool.tile(shape, F32, name=f"{pfx}_fl", tag=f"{pfx}_fl")
    nc.vector.tensor_tensor(fl[:], xf[:], gt[:], mybir.AluOpType.subtract)
    return fl


def build_program():
    import concourse.bass as bass
    import concourse.bacc as bacc
    import concourse.mybir as mybir
    import concourse.tile as tile
    from contextlib import ExitStack

    _patch_compat(bass)

    F32 = mybir.dt.float32
    F32R = mybir.dt.float32r
    BF16 = mybir.dt.bfloat16
    I16 = mybir.dt.int16
    TT = mybir.AluOpType
    ACTF = mybir.ActivationFunctionType

    nc = bacc.Bacc()

    # ---- external tensors ----
    qT_e = nc.declare_dram_parameter("qT", [C, LQ], F32, isOutput=False)
    memory_e = nc.declare_dram_parameter("memory", [HW, C], F32, isOutput=False)
    refpts_e = nc.declare_dram_parameter("refpts", [LQ, 2], F32, isOutput=False)
    w_value_e = nc.declare_dram_parameter("w_value", [C, C], BF16, isOutput=False)
    w_oa_e = nc.declare_dram_parameter("w_oa", [C, 96], F32, isOutput=False)
    b_oa_e = nc.declare_dram_parameter("b_oa", [128, 96], F32, isOutput=False)
    w_out_e = nc.declare_dram_parameter("w_out", [C, C], F32R, isOutput=False)
    b_out_e = nc.declare_dram_parameter("b_out", [128, C], F32, isOutput=False)
    b_val_e = nc.declare_dram_parameter("b_val", [128, C], F32, isOutput=False)
    ident_e = nc.declare_dram_parameter("ident", [128, 128], F32, isOutput=False)
    identb_e = nc.declare_dram_parameter("identb", [128, 128], BF16, isOutput=False)
    out_e = nc.declare_dram_parameter("out", [LQ, C], F32, isOutput=True)

    # internal DRAM: value as head-pairs [4][PAIR_ROWS, 64] bf16
    value_d = nc.dram_tensor("value_pairs", [4, PAIR_ROWS, 64], BF16)

    with tile.TileContext(nc) as tc, ExitStack() as ctx:
        cpool = ctx.enter_context(tc.tile_pool(name="const", bufs=1))
        qpool = ctx.enter_context(tc.tile_pool(name="qp", bufs=2))
        wpool = ctx.enter_context(tc.tile_pool(name="wm", bufs=1))
        mpool = ctx.enter_context(tc.tile_pool(name="mem", bufs=3))
        vpool = ctx.enter_context(tc.tile_pool(name="val", bufs=3))
        gpool = ctx.enter_context(tc.tile_pool(name="gat", bufs=4))
        spool = ctx.enter_context(tc.tile_pool(name="scr", bufs=2))
        psA = ctx.enter_context(tc.tile_pool(name="psA", bufs=3, space="PSUM"))
        psB = ctx.enter_context(tc.tile_pool(name="psB", bufs=2, space="PSUM"))
        _ps_n = [0]

        def ps_tr():  # [128,128] f32 transpose target, shared slots
            _ps_n[0] += 1
            return psA.tile([128, 128], F32, name=f"pstr{_ps_n[0]}", tag="pstr")

        def ps_trb():  # bf16 transpose target
            _ps_n[0] += 1
            return psA.tile([128, 128], BF16, name=f"psb{_ps_n[0]}", tag="psb")

        def ps_mm():  # [128,256] f32 matmul target
            _ps_n[0] += 1
            return psB.tile([128, C], F32, name=f"psmm{_ps_n[0]}", tag="psmm")

        ident = cpool.tile([128, 128], F32)
        nc.sync.dma_start(ident[:], ident_e[:])
        identb = cpool.tile([128, 128], BF16)
        nc.sync.dma_start(identb[:], identb_e[:])

        # ---------- 1. queryT (host pre-transposed) + projections ----------
        qTt = cpool.tile([128, 2, LQ], F32)
        nc.sync.dma_start(qTt[:], qT_e[:].rearrange("(k p) q -> p k q", k=2))
        qT = [qTt[:, 0, :], qTt[:, 1, :]]

        w_oa = cpool.tile([128, 2, 96], F32)
        nc.sync.dma_start(w_oa[:], w_oa_e[:].rearrange("(k p) o -> p k o", k=2))
        b_oa = cpool.tile([128, 96], F32)
        nc.sync.dma_start(b_oa[:], b_oa_e[:])

        # off_all [128, 8, 96] natural layout (partition = q%128, qtop free)
        off_all = cpool.tile([128, 8, 96], F32)
        for qc in range(8):
            po = ps_mm()
            for kc in range(2):
                nc.tensor.matmul(po[:, 0:96], qT[kc][:, qc * 128:(qc + 1) * 128],
                                 w_oa[:, kc, :], start=(kc == 0), stop=(kc == 1))
            nc.vector.tensor_tensor(off_all[:, qc, :], po[:, 0:96], b_oa[:], TT.add)

        # refpts natural [128, 8, 2]
        refs = cpool.tile([128, 8, 2], F32)
        nc.sync.dma_start(
            refs[:], refpts_e[:].rearrange("(g p) t -> p g t", p=128))

        # ---------- 2. weight math ----------
        S8 = [128, 8, 32]      # (q%128, qtop, (h, pt))

        def view_off(comp):  # comp 0 = x, 1 = y -> [128, 8, 8, 4] strided view
            return off_all[:, :, comp:64 + comp].rearrange(
                "p g (h pt two) -> p g h pt two", h=8, two=2)[:, :, :, :, 0]

        wm = ctx.enter_context(tc.tile_pool(name="wmath", bufs=1))

        _wm_n = [0]

        def ttile():
            _wm_n[0] += 1
            nm = f"wmath{_wm_n[0]}"
            return wm.tile(S8, F32, name=nm, tag=nm)

        # --- x chain ---
        px = ttile()
        # px = (ref_x + ox/128) * 128 - 0.5   (matches reference rounding)
        nc.vector.tensor_scalar(px[:], view_off(0), 1.0 / GRID, None, TT.mult)
        nc.vector.tensor_tensor(
            px[:], px[:], refs[:, :, 0:1].broadcast_to(S8), TT.add)
        nc.vector.tensor_scalar(px[:], px[:], float(GRID), -0.5, TT.mult, TT.add)
        x0 = _floor(nc, wm, mybir, px, S8, "fx0")
        wx1 = ttile()
        nc.vector.tensor_tensor(wx1[:], px[:], x0[:], TT.subtract)
        wx0 = ttile()
        nc.vector.tensor_scalar(wx0[:], wx1[:], -1.0, 1.0, TT.mult, TT.add)
        ge0 = ttile()
        nc.vector.tensor_scalar(ge0[:], x0[:], 0.0, None, TT.is_ge)
        le127 = ttile()
        nc.vector.tensor_scalar(le127[:], x0[:], 127.0, None, TT.is_le)
        le126 = ttile()
        nc.vector.tensor_scalar(le126[:], x0[:], 126.0, None, TT.is_le)
        eqm1 = ttile()
        nc.vector.tensor_scalar(eqm1[:], x0[:], -1.0, None, TT.is_equal)
        # c0 = wx0*inb(x0) + wx1*(x0 == -1);  c1 = wx1*(0 <= x0 <= 126)
        c0 = ttile()
        nc.vector.tensor_tensor(c0[:], ge0[:], le127[:], TT.mult)
        nc.vector.tensor_tensor(c0[:], c0[:], wx0[:], TT.mult)
        t_ = ttile()
        nc.vector.tensor_tensor(t_[:], wx1[:], eqm1[:], TT.mult)
        nc.vector.tensor_tensor(c0[:], c0[:], t_[:], TT.add)
        c1 = ttile()
        nc.vector.tensor_tensor(c1[:], ge0[:], le126[:], TT.mult)
        nc.vector.tensor_tensor(c1[:], c1[:], wx1[:], TT.mult)
        # xs = clip(x0, 0, 127); kh = floor(xs/2); s = xs - 2*kh
        xs = ttile()
        nc.vector.tensor_scalar(xs[:], x0[:], 0.0, 127.0, TT.max, TT.min)
        xh = ttile()
        nc.vector.tensor_scalar(xh[:], xs[:], 0.5, None, TT.mult)
        kh = _floor(nc, wm, mybir, xh, S8, "fkh")
        spar = ttile()
        nc.vector.scalar_tensor_tensor(spar[:], kh[:], -2.0, xs[:], TT.mult, TT.add)

        # --- y chain ---
        py = ttile()
        nc.vector.tensor_scalar(py[:], view_off(1), 1.0 / GRID, None, TT.mult)
        nc.vector.tensor_tensor(
            py[:], py[:], refs[:, :, 1:2].broadcast_to(S8), TT.add)
        nc.vector.tensor_scalar(py[:], py[:], float(GRID), -0.5, TT.mult, TT.add)
        y0 = _floor(nc, wm, mybir, py, S8, "fy0")
        wy1 = ttile()
        nc.vector.tensor_tensor(wy1[:], py[:], y0[:], TT.subtract)
        wy0m = ttile()
        nc.vector.tensor_scalar(wy0m[:], wy1[:], -1.0, 1.0, TT.mult, TT.add)
        yge0 = ttile()
        nc.vector.tensor_scalar(yge0[:], y0[:], 0.0, None, TT.is_ge)
        yle127 = ttile()
        nc.vector.tensor_scalar(yle127[:], y0[:], 127.0, None, TT.is_le)
        nc.vector.tensor_tensor(yge0[:], yge0[:], yle127[:], TT.mult)
        nc.vector.tensor_tensor(wy0m[:], wy0m[:], yge0[:], TT.mult)  # wy0*inb(y0)
        ygem1 = ttile()
        nc.vector.tensor_scalar(ygem1[:], y0[:], -1.0, None, TT.is_ge)
        yle126 = ttile()
        nc.vector.tensor_scalar(yle126[:], y0[:], 126.0, None, TT.is_le)
        nc.vector.tensor_tensor(ygem1[:], ygem1[:], yle126[:], TT.mult)
        nc.vector.tensor_tensor(wy1[:], wy1[:], ygem1[:], TT.mult)   # wy1*inb(y1)
        r0 = ttile()
        nc.vector.tensor_scalar(r0[:], y0[:], 0.0, 127.0, TT.max, TT.min)
        r1 = ttile()
        nc.vector.tensor_scalar(r1[:], y0[:], 1.0, None, TT.add)
        nc.vector.tensor_scalar(r1[:], r1[:], 0.0, 127.0, TT.max, TT.min)

        # --- softmax over pt ---
        logit4 = off_all[:, :, 64:96].rearrange("p g (h pt) -> p g h pt", pt=4)
        mx = wm.tile([128, 8, 8], F32, name="smx", tag="smx")
        nc.vector.tensor_reduce(mx[:], logit4, mybir.AxisListType.X, TT.max)
        ee = ttile()
        nc.vector.tensor_tensor(
            ee[:].rearrange("p g (h pt) -> p g h pt", pt=4), logit4,
            mx[:].unsqueeze(3).broadcast_to([128, 8, 8, 4]),
            TT.subtract)
        nc.scalar.activation(ee[:], ee[:], ACTF.Exp)
        ssum = wm.tile([128, 8, 8], F32, name="ssum", tag="ssum")
        nc.vector.tensor_reduce(
            ssum[:], ee[:].rearrange("p g (h pt) -> p g h pt", pt=4),
            mybir.AxisListType.X, TT.add)
        rec = wm.tile([128, 8, 8], F32, name="srec", tag="srec")
        nc.vector.reciprocal(rec[:], ssum[:])
        attn = ttile()
        nc.vector.tensor_tensor(
            attn[:].rearrange("p g (h pt) -> p g h pt", pt=4),
            ee[:].rearrange("p g (h pt) -> p g h pt", pt=4),
            rec[:].unsqueeze(3).broadcast_to([128, 8, 8, 4]),
            TT.mult)

        # --- fused coefficients C[y][j] = attn * wy_y_masked * v_j ---
        # v0 = c0*(s==0), v1 = c0*(s==1)+c1*(s==0), v2 = c1*(s==1)
        p0 = ttile()
        nc.vector.tensor_scalar(p0[:], spar[:], 0.0, None, TT.is_equal)
        p1 = ttile()
        nc.vector.tensor_scalar(p1[:], spar[:], 1.0, None, TT.is_equal)
        g0 = ttile()
        nc.vector.tensor_tensor(g0[:], attn[:], wy0m[:], TT.mult)
        g1 = ttile()
        nc.vector.tensor_tensor(g1[:], attn[:], wy1[:], TT.mult)
        v0 = ttile()
        nc.vector.tensor_tensor(v0[:], c0[:], p0[:], TT.mult)
        v1 = ttile()
        nc.vector.tensor_tensor(v1[:], c0[:], p1[:], TT.mult)
        nc.vector.tensor_tensor(t_[:], c1[:], p0[:], TT.mult)
        nc.vector.tensor_tensor(v1[:], v1[:], t_[:], TT.add)
        v2 = ttile()
        nc.vector.tensor_tensor(v2[:], c1[:], p1[:], TT.mult)
        # coeff tensor [128, qtop, y, j, h, pt] fp32
        coef = cpool.tile([128, 8, 2, 3, 8, 4], F32)
        for yi, gy in ((0, g0), (1, g1)):
            for ji, vj in ((0, v0), (1, v1), (2, v2)):
                nc.vector.tensor_tensor(
                    coef[:, :, yi, ji, :, :],
                    gy[:].rearrange("p g (h pt) -> p g h pt", pt=4),
                    vj[:].rearrange("p g (h pt) -> p g h pt", pt=4), TT.mult)

        # bias-fold factor S[q, h] = sum_pt attn*(wy0m+wy1m)*(c0+c1)
        wys = ttile()
        nc.vector.tensor_tensor(wys[:], wy0m[:], wy1[:], TT.add)
        cxs = ttile()
        nc.vector.tensor_tensor(cxs[:], c0[:], c1[:], TT.add)
        nc.vector.tensor_tensor(wys[:], wys[:], cxs[:], TT.mult)
        nc.vector.tensor_tensor(wys[:], wys[:], attn[:], TT.mult)
        sfac = cpool.tile([128, 8, 8], F32)
        nc.vector.tensor_reduce(
            sfac[:], wys[:].rearrange("p g (h pt) -> p g h pt", pt=4),
            mybir.AxisListType.X, TT.add)

        # --- gather window indices idxf [128, (qtop, h, pt, y)] fp32 ---
        idxf = cpool.tile([128, 8, 8, 4, 2], F32)
        for yi, rr in ((0, r0), (1, r1)):
            nc.vector.scalar_tensor_tensor(
                idxf[:, :, :, :, yi],
                rr[:].rearrange("p g (h pt) -> p g h pt", pt=4), 64.0,
                kh[:].rearrange("p g (h pt) -> p g h pt", pt=4),
                TT.mult, TT.add)

        # ---------- idx layout transform: [128, 512] -> [16, 64, 64] ----------
        # T1: 4 PE transposes -> T-all [4][128 f, 128 q%128]
        tall = [cpool.tile([128, 128], F32, name=f"tall{t}", tag=f"tall{t}")
                for t in range(4)]
        idxf_flat = idxf[:].rearrange("p g h pt y -> p (g h pt y)")
        for t in range(4):
            pt_ = ps_tr()
            nc.tensor.transpose(
                pt_[:], idxf_flat[:, t * 128:(t + 1) * 128], ident[:])
            nc.scalar.copy(tall[t][:], pt_[:])
        # T2: per (t, qmid): [128 f, 16] -> [16, 128 f]; scatter into IDX
        idx_f2 = cpool.tile([16, 64, 8, 8], F32)  # [qlo, call, qtop, qmid]
        for t in range(4):
            for qmid in range(8):
                ptf = ps_tr()
                pt_ = ptf[0:16, :]
                nc.tensor.transpose(
                    pt_, tall[t][:, qmid * 16:qmid * 16 + 16], ident[:])
                # f = t*128 + j, j = (qtop%2)*64 + call ; qtop = 2t + (j//64)
                src = pt_.rearrange("a (q2 c) -> a q2 c", q2=2)
                nc.scalar.copy(
                    idx_f2[:, :, 2 * t:2 * t + 2, qmid].rearrange(
                        "a c q2 -> a q2 c"), src)
        idx16 = cpool.tile([16, 64 * 64], I16)
        nc.vector.tensor_copy(
            idx16[:], idx_f2[:].rearrange("a c g q -> a (c g q)"))
        idxr = cpool.tile([128, 64, 64], I16)
        for rep in range(8):
            nc.sync.dma_start(
                idxr[rep * 16:(rep + 1) * 16, :, :],
                idx16[:].rearrange("a (c b) -> a c b", c=64))

        # ---------- 3. value projection ----------
        w_val = cpool.tile([128, 2, C], BF16)
        nc.sync.dma_start(w_val[:], w_value_e[:].rearrange(
            "(k p) o -> p k o", k=2))
        zpad = cpool.tile([2, 64], BF16)
        nc.vector.memset(zpad[:], 0.0)
        for pr in range(4):
            nc.sync.dma_start(value_d[pr, HW:HW + 2, :], zpad[:])

        for mc8 in range(16):
            mt8 = mpool.tile([128, 8, C], BF16, name="mt8", tag="mt8")
            nc.gpsimd.dma_start(
                mt8[:], memory_e[mc8 * 1024:(mc8 + 1) * 1024, :].rearrange(
                    "(i p) c -> p i c", p=128))
            for grp in range(2):
                vt4 = vpool.tile([128, 4, C], BF16, name="vt4", tag="vt4")
                for i in range(4):
                    sub = grp * 4 + i
                    mT = mpool.tile([128, 2, 128], BF16, name="mT", tag="mT")
                    for kc in range(2):
                        pt_ = ps_trb()
                        nc.tensor.transpose(
                            pt_[:], mt8[:, sub, kc * 128:(kc + 1) * 128],
                            identb[:])
                        nc.scalar.copy(mT[:, kc, :], pt_[:])
                    pv = ps_mm()
                    for kc in range(2):
                        nc.tensor.matmul(pv[:], mT[:, kc, :], w_val[:, kc, :],
                                         start=(kc == 0), stop=(kc == 1))
                    if i % 2 == 0:
                        nc.scalar.copy(vt4[:, i, :], pv[:])
                    else:
                        nc.vector.tensor_copy(vt4[:, i, :], pv[:])
                r0_ = (mc8 * 8 + grp * 4) * 128
                for pr in range(4):
                    eng = nc.sync if pr % 2 == 0 else nc.scalar
                    eng.dma_start(
                        value_d[pr, r0_:r0_ + 512, :].rearrange(
                            "(i p) c -> p i c", p=128),
                        vt4[:, :, pr * 64:(pr + 1) * 64])

        # ---------- 4 & 5. gather + blend ----------
        sampled = cpool.tile([128, 8, 8, 32], F32)  # [q%128, qtop, h, c]
        val_flat = value_d[:].rearrange("pr r c -> (pr r c)")
        for h in range(NH):
            pr = h // 2
            half = h % 2
            base = pr * (PAIR_ROWS * 64)
            in_ap = val_flat[base:base + 8192 * 128].rearrange(
                "(n c) -> n c", c=128).copy()
            in_ap.ap[-1] = (1, 256)  # overlapping 256-elem windows, step 128
            acc = spool.tile([128, 8, 32], F32, tag="acc")
            first = True
            for pt_i in range(NPT):
                for yi in range(2):
                    call = ((h * NPT) + pt_i) * 2 + yi
                    gat = gpool.tile([128, 8, 256], BF16)
                    nc.gpsimd.dma_gather(
                        gat[:], in_ap, idxr[:, call, :], LQ, LQ, 256,
                        elem_step=128)
                    sc = gpool.tile([128, 8, 3, 32], F32, tag="scaled")
                    g3 = gat[:].rearrange("p g (j c) -> p g j c", c=64)[
                        :, :, 0:3, half * 32:half * 32 + 32]
                    cf = coef[:, :, yi, :, h, pt_i].unsqueeze(3).broadcast_to([128, 8, 3, 32])
                    nc.vector.tensor_tensor(sc[:], g3, cf, TT.mult)
                    red = gpool.tile([128, 8, 32], F32, tag="red")
                    nc.vector.tensor_reduce(
                        red[:], sc[:].rearrange("p g j c -> p g c j"),
                        mybir.AxisListType.X, TT.add)
                    if first:
                        nc.vector.tensor_copy(acc[:], red[:])
                        first = False
                    else:
                        nc.vector.tensor_tensor(acc[:], acc[:], red[:], TT.add)
            nc.vector.tensor_copy(sampled[:, :, h, :], acc[:])

        # bias fold: sampled += S[q, h] * b_value[h*32 + c]
        b_val = cpool.tile([128, C], F32)
        nc.sync.dma_start(b_val[:], b_val_e[:])
        bterm = spool.tile([128, 8, 8, 32], F32, tag="bterm")
        nc.vector.tensor_tensor(
            bterm[:],
            sfac[:].unsqueeze(3).broadcast_to([128, 8, 8, 32]),
            b_val[:].rearrange("p (h c) -> p h c", h=8).unsqueeze(1).broadcast_to(
                [128, 8, 8, 32]),
            TT.mult)
        nc.vector.tensor_tensor(sampled[:], sampled[:], bterm[:], TT.add)

        # ---------- 6. output projection ----------
        # sampledT [2][128 hc, (qtop, q%128)] f32r
        sT = [cpool.tile([128, 8, 128], F32R, name=f"sT{i}", tag=f"sT{i}")
              for i in range(2)]
        for qt_ in range(8):
            for hf in range(2):
                pt_ = ps_tr()
                nc.tensor.transpose(
                    pt_[:],
                    sampled[:, qt_, hf * 4:(hf + 1) * 4, :].rearrange(
                        "p h c -> p (h c)"),
                    ident[:])
                nc.scalar.copy(sT[hf][:, qt_, :], pt_[:])
        w_out = cpool.tile([128, 2, C], F32R)
        nc.sync.dma_start(w_out[:], w_out_e[:].rearrange(
            "(k p) o -> p k o", k=2))
        b_out = cpool.tile([128, C], F32)
        nc.sync.dma_start(b_out[:], b_out_e[:])
        for qt_ in range(8):
            po = ps_mm()
            for kc in range(2):
                nc.tensor.matmul(po[:], sT[kc][:, qt_, :], w_out[:, kc, :],
                                 start=(kc == 0), stop=(kc == 1))
            ot = qpool.tile([128, C], F32, tag="out")
            nc.vector.tensor_tensor(ot[:], po[:], b_out[:], TT.add)
            nc.sync.dma_start(out_e[qt_ * 128:(qt_ + 1) * 128, :], ot[:])

    nc.finalize()
    return nc


_CACHE = {}


def _get_program():
    if "nc" not in _CACHE:
        _CACHE["nc"] = build_program()
    return _CACHE["nc"]


def run(inputs, trace=False):
    from concourse.bass_utils import run_bass_kernel_spmd

    nc = _get_program()
    query = np.asarray(inputs["query"], np.float32)
    memory = np.asarray(inputs["memory"], np.float32)
    refpts = np.asarray(inputs["reference_points"], np.float32)
    w_value = np.asarray(inputs["W_value"], np.float32).astype(ml_dtypes.bfloat16)
    b_value = np.asarray(inputs["b_value"], np.float32)
    w_off = np.asarray(inputs["W_off"], np.float32)
    b_off = np.asarray(inputs["b_off"], np.float32)
    w_attn = np.asarray(inputs["W_attn"], np.float32)
    b_attn = np.asarray(inputs["b_attn"], np.float32)
    w_out = np.asarray(inputs["W_out"], np.float32)
    b_out = np.asarray(inputs["b_out"], np.float32)

    w_oa = np.concatenate([w_off, w_attn], axis=1).astype(np.float32)
    b_oa = np.tile(np.concatenate([b_off, b_attn])[None, :], (128, 1)).astype(
        np.float32)
    b_out_r = np.tile(b_out[None, :], (128, 1)).astype(np.float32)
    b_val_r = np.tile(b_value[None, :], (128, 1)).astype(np.float32)
    ident = np.eye(128, dtype=np.float32)
    identb = np.eye(128, dtype=ml_dtypes.bfloat16)

    shared = dict(w_value=w_value, w_oa=w_oa, b_oa=b_oa, w_out=w_out,
                  b_out=b_out_r, b_val=b_val_r, ident=ident, identb=identb)
    in_maps = []
    for i in range(N_B):
        m = dict(shared)
        m["qT"] = np.ascontiguousarray(query[i].T)
        m["memory"] = memory[i]
        m["refpts"] = refpts[i]
        in_maps.append(m)

    res = run_bass_kernel_spmd(nc, in_maps, list(range(N_B)), trace=trace,
                               trace_cores=[0])
    out = np.stack([res.results[i]["out"] for i in range(N_B)], axis=0)
    return out, res


def kernel(**inputs):
    assert int(inputs.get("H", GRID)) == GRID and int(inputs.get("W", GRID)) == GRID
    out, _ = run(inputs, trace=False)
    return out.astype(np.float32)

